# revision 1
# baseline (speedup 1.0000x reference)
"""AreaAttention Trainium2 kernel: B=8 data-parallel over 8 NeuronCores.

Reference computation (per sample, C=128 channels, N=H*W=4096 pixels):
    q = Wq@x + bq                    ('oc,bcn->bno' proper matmul)
    k = x * colsum(Wk) + bk          ('oc,bcn->bcn' keeps c: per-channel scale!)
    v = Wv@x + bv                    ('oc,bcn->bno')
    out = x + softmax(q^T k / sqrt(C)) @ v^T

Per-core design (one sample per core, no collectives):
  - q16/k16 stored [c, n] fp16; v16 stored [m, c] fp16 (PSUM accumulates fp32).
  - Scores computed TRANSPOSED: sT[m, n] = k_chunk^T @ q  (free dim 512).
  - exp((s*scale)+E) with global shift E=+1 (cancels in softmax). Most pairs
    on ScalarE (ACT); DVE_PAIRS per block are computed on the Vector engine
    via a Schraudolph bit-trick: fp16_bits = round(A*(s*scale+E)+B) emitted
    by one tensor_scalar into a uint16 view (rms rel err ~1.8%, verified on
    HW). ACT and DVE exps run CONCURRENTLY on different PSUM banks, lifting
    the per-block exp throughput above either engine alone.
  - PV: out[c, n] += v_chunk^T @ expS  -> output directly in [c, n] layout.
  - Softmax denominator: 2 fp16 partial-sum chains over chunk-pairs, ones-
    matmul reduces partitions AND broadcasts row-sums; reciprocal via the
    single-instruction reciprocal_approx_fast (fp32, ~51 ULP), multiply,
    +residual.
  - Setup offload: xf16 casts / k16 scale / residual adds run on the idle
    GpSimd engine; q-projections are spread across blocks (q slice s+1 is
    produced during block s) to unclog DVE in block 0.
"""
import numpy as np

C = 128
N = 4096          # 64*64
NB = 1024         # n-block span
NBLK = N // NB    # 4
MCH = N // C      # 32 m-chunks
NPAIR = MCH // 2  # 16 chunk-pairs per block
SCALE = 1.0 / np.sqrt(np.float32(C))
ESHIFT = 1.0      # global softmax shift: es = exp(s*scale + E); cancels.

# Schraudolph fp16 exp: bits = round(A16*(arg) + B16); bitcast -> fp16 ~ e^arg
A16 = 1477.319722
B16 = 15300.5
DVE_MULT = float(A16 * SCALE)
DVE_ADD = float(A16 * ESHIFT + B16)
# pairs computed on DVE instead of ACT, per block (avoid the hoisted pairs
# {0,1,2} and block-0's setup jps; block 0's DVE is busier with v16/qproj)
DVE_PAIRS = {0: (6, 10, 13), 1: (6, 12), 2: (6, 12), 3: (6, 12)}
# every block's last pair: chunk u=0 on ACT, u=1 on DVE, so both exps
# finish in parallel and the boundary chain/rowsum path starts earlier

_cache = {}


def _build_nc():
    import concourse.tile as tile
    from concourse import bacc, mybir

    f32 = mybir.dt.float32
    f16 = mybir.dt.float16
    u16 = mybir.dt.uint16
    ADD = mybir.AluOpType.add
    MUL = mybir.AluOpType.mult
    EXP = mybir.ActivationFunctionType.Exp

    nc = bacc.Bacc("TRN2", target_bir_lowering=False)

    x_d = nc.dram_tensor("x", [C, N], f32, kind="ExternalInput")
    # packed weights: one tiny DMA each instead of six serialized ones
    wqv16_d = nc.dram_tensor("wqv16", [C, 2 * C], f16, kind="ExternalInput")
    smalls_d = nc.dram_tensor("smalls", [C, 4], f32, kind="ExternalInput")
    out_d = nc.dram_tensor("out", [C, N], f32, kind="ExternalOutput")

    with tile.TileContext(nc) as tc:
        with tc.tile_pool(name="big", bufs=1) as big, \
             tc.tile_pool(name="small", bufs=1) as small, \
             tc.tile_pool(name="es_pool", bufs=8) as es_pool, \
             tc.tile_pool(name="p_pool", bufs=6) as p_pool, \
             tc.tile_pool(name="work", bufs=2) as work, \
             tc.tile_pool(name="ps_sc", bufs=3, space="PSUM") as ps_sc, \
             tc.tile_pool(name="ps_pv", bufs=1, space="PSUM") as ps_pv:

            xfb = big.tile([C, N], f32, tag="xfb")      # x, then x + bv (residual)
            xf16 = big.tile([C, N], f16, tag="xf16")    # x fp16 (q/v proj, k build)
            q16 = big.tile([C, N], f16, tag="q16")
            k16 = big.tile([C, N], f16, tag="k16")
            v16 = big.tile([C, N], f16, tag="v16")      # chunk j at cols [128j,128j+128) = v[m, c]

            wqv16 = small.tile([C, 2 * C], f16, tag="wqv16")
            smalls = small.tile([C, 4], f32, tag="smalls")
            ebias = small.tile([C, 1], f32, tag="ebias")
            ones16 = small.tile([C, C], f16, tag="ones16")
            wqt16 = wqv16[:, 0:C]
            wvt16 = wqv16[:, C:2 * C]
            wks = smalls[:, 0:1]
            bk = smalls[:, 1:2]
            bq = smalls[:, 2:3]
            bv = smalls[:, 3:4]

            # Per-queue DMA bandwidth is ~50GB/s, so spread the loads: slice 0
            # split across sync+vector queues (it gates the first QK), the
            # small weights FIRST on the scalar queue (they gate qproj/k16 and
            # would otherwise queue behind megabytes of x), bulk x on gpsimd.
            nc.sync.dma_start(xfb[:, 0:512], x_d[:, 0:512])
            nc.sync.dma_start(smalls[:], smalls_d[:])
            nc.scalar.dma_start(xfb[:, 512:NB], x_d[:, 512:NB])
            nc.scalar.dma_start(wqv16[:], wqv16_d[:])
            nc.sync.dma_start(xfb[:, NB:2 * NB], x_d[:, NB:2 * NB])
            nc.scalar.dma_start(xfb[:, 2 * NB:3 * NB], x_d[:, 2 * NB:3 * NB])
            nc.sync.dma_start(xfb[:, 3 * NB:4 * NB], x_d[:, 3 * NB:4 * NB])
            nc.vector.memset(ebias[:], ESHIFT)
            nc.vector.memset(ones16[:], 1.0)

            def cast_k(s):
                """xf16 cast + k16 build for one slice (DVE)."""
                sl = slice(s * NB, (s + 1) * NB)
                nc.vector.tensor_copy(xf16[:, sl], xfb[:, sl])
                nc.vector.tensor_scalar(k16[:, sl], xf16[:, sl], wks, bk,
                                        op0=MUL, op1=ADD)

            def qproj(s):
                """q16 for one 1024-col slice (2 matmuls + DVE bias add)."""
                sl = slice(s * NB, (s + 1) * NB)
                ps = ps_sc.tile([C, NB], f32, tag="sc", name=f"qps{s}")
                for h in range(2):
                    hsl = slice(s * NB + h * 512, s * NB + (h + 1) * 512)
                    nc.tensor.matmul(ps[:, h * 512:(h + 1) * 512], wqt16,
                                     xf16[:, hsl], start=True, stop=True)
                nc.vector.tensor_scalar(q16[:, sl], ps[:], bq, None, op0=ADD)

            def setup_v(s):
                """v16 chunks for one 1024-col slice (8 chunks, one psum tile)."""
                sl = slice(s * NB, (s + 1) * NB)
                psv = ps_sc.tile([C, NB], f32, tag="sc", name=f"vps{s}")
                for t in range(8):
                    j = s * 8 + t
                    nc.tensor.matmul(psv[:, t * C:(t + 1) * C],
                                     xf16[:, j * C:(j + 1) * C], wvt16,
                                     start=True, stop=True)
                nc.vector.tensor_copy(v16[:, sl], psv[:])

            def resid_slice(s):
                sl = slice(s * NB, (s + 1) * NB)
                nc.vector.tensor_scalar(xfb[:, sl], xfb[:, sl], bv, None, op0=ADD)

            def tail(nb, pv, chains, last_es2):
                n0 = nb * NB
                last = nb == NBLK - 1
                # row-sum over partitions; the ones[128,128] stationary operand
                # broadcasts the sum to all partitions. Per-half rs tiles for
                # EVERY block: half 0's reciprocal overlaps half 1's rowsum
                # matmuls and each PSUM slot is released as soon as its half
                # is read (the next block's QKs recycle these buffers).
                rs_t = [ps_sc.tile([C, 512], f32, tag="sc", name=f"rs{nb}_{h}")
                        for h in range(2)]
                srcs = [chains[0], chains[1]]
                if last_es2 is not None:
                    srcs.append(last_es2)
                rb = work.tile([C, NB], f32, tag="rb", name=f"rb{nb}")
                ep1 = work.tile([C, NB], f32, tag="ep1", name=f"ep{nb}")
                ost = work.tile([C, NB], f32, tag="ost", name=f"ost{nb}")
                for h in range(2):
                    hsl = slice(h * 512, (h + 1) * 512)
                    for ci, src in enumerate(srcs):
                        dst = rs_t[h][:, 0:512]
                        nc.tensor.matmul(dst, ones16[:], src[:, hsl],
                                         start=(ci == 0), stop=False)
                        nc.tensor.matmul(dst, ones16[:],
                                         src[:, NB + h * 512:NB + (h + 1) * 512],
                                         start=False, stop=(ci == len(srcs) - 1))
                    nc.vector.reciprocal_approx_fast(out=rb[:, hsl],
                                                     in_=rs_t[h][:, 0:512])
                    if last:
                        # per-half normalize, quarter-grain residual+DMA so
                        # the kernel's final transfer is small and early
                        nc.vector.tensor_tensor(ep1[:, hsl], pv[:, hsl],
                                                rb[:, hsl], op=MUL)
                        for qi, eng in enumerate((nc.sync, nc.scalar)):
                            a = h * 512 + qi * 256
                            qsl = slice(a, a + 256)
                            nc.vector.tensor_tensor(ost[:, qsl], ep1[:, qsl],
                                                    xfb[:, n0 + a:n0 + a + 256],
                                                    op=ADD)
                            eng.dma_start(out_d[:, n0 + a:n0 + a + 256],
                                          ost[:, qsl])
                if not last:
                    nc.vector.tensor_tensor(ep1[:], pv[:], rb[:], op=MUL)
                    nc.vector.tensor_tensor(ost[:], ep1[:], xfb[:, n0:n0 + NB],
                                            op=ADD)
                    nc.sync.dma_start(out_d[:, n0:n0 + NB], ost[:])

            # slice-0 setup at half granularity so the first QK/exp is not
            # gated on the second x-half DMA; later slices' casts are
            # interleaved into block 0 (setup_v needs xf16 of every slice).
            for h in range(2):
                hsl = slice(h * 512, (h + 1) * 512)
                qps0 = ps_sc.tile([C, 512], f32, tag="sc", name=f"qps0_{h}")
                nc.vector.tensor_copy(xf16[:, hsl], xfb[:, hsl])
                nc.vector.tensor_scalar(k16[:, hsl], xf16[:, hsl], wks, bk,
                                        op0=MUL, op1=ADD)
                nc.tensor.matmul(qps0[:, 0:512], wqt16, xf16[:, hsl],
                                 start=True, stop=True)
                # bias add on the (still idle) Scalar engine to unclog DVE
                nc.scalar.add(q16[:, hsl], qps0[:, 0:512], bq)

            def emit_qk_exp(nb, jp):
                """QK matmuls + exp for one chunk-pair; returns the es2 tile.

                ACT pairs: scalar.activation Exp (exact). DVE_PAIRS: one
                vector.tensor_scalar per chunk writing Schraudolph fp16 bits
                through a uint16 view (runs concurrently with ACT pairs).
                """
                n0 = nb * NB
                es2 = es_pool.tile([C, 2 * NB], f16, tag="es", name=f"es{nb}_{jp}")
                # last pair: DVE chunk (u=1) first so its exp starts two
                # matmuls earlier; chain(15) then closes sooner
                for u in ((1, 0) if jp == NPAIR - 1 else (0, 1)):
                    on_dve = (jp in DVE_PAIRS[nb]) or (jp == NPAIR - 1 and u == 1)
                    j = 2 * jp + u
                    ksl = slice(j * C, (j + 1) * C)
                    sc = ps_sc.tile([C, NB], f32, tag="sc", name=f"sc{nb}_{j}")
                    for h in range(2):
                        nc.tensor.matmul(sc[:, h * 512:(h + 1) * 512],
                                         k16[:, ksl],
                                         q16[:, n0 + h * 512:n0 + (h + 1) * 512],
                                         start=True, stop=True)
                    usl = slice(u * NB, (u + 1) * NB)
                    if on_dve:
                        nc.vector.tensor_scalar(es2[:, usl].bitcast(u16), sc[:],
                                                DVE_MULT, DVE_ADD,
                                                op0=MUL, op1=ADD)
                    elif nb == 0 and jp == 0:
                        # very first pair: per-512 activations so the first
                        # exp starts before the second QK half lands
                        for h in range(2):
                            ql = slice(u * NB + h * 512, u * NB + (h + 1) * 512)
                            nc.scalar.activation(es2[:, ql],
                                                 sc[:, h * 512:(h + 1) * 512],
                                                 EXP, bias=ebias[:],
                                                 scale=float(SCALE))
                    else:
                        nc.scalar.activation(es2[:, usl], sc[:], EXP,
                                             bias=ebias[:], scale=float(SCALE))
                return es2

            def emit_pv(nb, jp, pv, es2):
                for u in range(2):
                    j = 2 * jp + u
                    ksl = slice(j * C, (j + 1) * C)
                    for h in range(2):
                        nc.tensor.matmul(pv[:, h * 512:(h + 1) * 512],
                                         v16[:, ksl],
                                         es2[:, u * NB + h * 512:u * NB + (h + 1) * 512],
                                         start=(j == 0), stop=(j == MCH - 1))

            NHOIST = 3  # next-block QK/exp pairs emitted ahead of each tail
            hoisted = {}
            for nb in range(NBLK):
                pv = ps_pv.tile([C, NB], f32, tag="pv", name=f"pv{nb}")
                chains = [p_pool.tile([C, 2 * NB], f16, tag="pacc", name=f"pacc{nb}_{i}")
                          for i in range(2)]
                pend = None  # deferred chain op: DVE exps jump ahead of it

                def flush_chain():
                    nonlocal pend
                    if pend is None:
                        return
                    jq, es = pend
                    pend = None
                    if jq < 2:
                        nc.vector.tensor_copy(chains[jq % 2][:], es[:])
                    else:
                        nc.vector.tensor_tensor(chains[jq % 2][:],
                                                chains[jq % 2][:], es[:], op=ADD)

                for jp in range(NPAIR):
                    if nb == 0 and jp in (2, 5, 9):
                        # emit setup early: block-0's DVE queue is congested,
                        # so the v16 copies must enter it well before the PV
                        # that consumes them
                        s = {2: 1, 5: 2, 9: 3}[jp]
                        cast_k(s)
                        setup_v(s)
                    if jp == 2 and nb < NBLK - 1:
                        qproj(nb + 1)           # q for the NEXT block
                    if jp == 9:
                        resid_slice(nb)         # xfb += bv for this block
                    es2 = hoisted.pop((nb, jp), None)
                    if es2 is None:
                        es2 = emit_qk_exp(nb, jp)
                    flush_chain()               # previous pair's chain op
                    if nb == 0 and jp == 0:
                        setup_v(0)
                    # hoist the NEXT block's first QK/exp pairs ahead of this
                    # block's final PVs + tail so ACT never waits at the boundary
                    if jp == NPAIR - 1 and nb < NBLK - 1:
                        for hj in range(NHOIST):
                            hoisted[(nb + 1, hj)] = emit_qk_exp(nb + 1, hj)
                    emit_pv(nb, jp, pv, es2)
                    # denominator: 2 interleaved fp16 chains over chunk-pairs,
                    # deferred one pair so a DVE-pair exp runs before it. In
                    # the FINAL block the last pair skips the chain -- its
                    # contribution goes straight into the rowsum matmuls.
                    if jp == NPAIR - 1 and nb == NBLK - 1:
                        last_es2 = es2
                    else:
                        pend = (jp, es2)

                flush_chain()
                tail(nb, pv, chains, last_es2 if nb == NBLK - 1 else None)

    nc.finalize()
    return nc


def _get_nc():
    if "nc" not in _cache:
        _cache["nc"] = _build_nc()
    return _cache["nc"]


def make_in_maps(x, Wq, bq, Wk, bk, Wv, bv):
    x = np.asarray(x, dtype=np.float32)
    B = x.shape[0]
    wqt16 = np.asarray(Wq, np.float32).T.astype(np.float16)
    wvt16 = np.asarray(Wv, np.float32).T.astype(np.float16)
    wqv16 = np.ascontiguousarray(np.concatenate([wqt16, wvt16], axis=1))
    wks = np.asarray(Wk, np.float32).sum(axis=0)
    smalls = np.ascontiguousarray(np.stack(
        [wks, np.asarray(bk, np.float32), np.asarray(bq, np.float32),
         np.asarray(bv, np.float32)], axis=1).astype(np.float32))
    in_maps = []
    for i in range(B):
        in_maps.append({
            "x": np.ascontiguousarray(x[i].reshape(C, N)),
            "wqv16": wqv16, "smalls": smalls,
        })
    return in_maps


def kernel(x, Wq, bq, Wk, bk, Wv, bv, _trace=False, _tmpdir=None):
    from concourse.bass_utils import run_bass_kernel_spmd

    x = np.asarray(x, dtype=np.float32)
    B, c, H, W = x.shape
    assert (c, H * W) == (C, N), (c, H, W)
    in_maps = make_in_maps(x, Wq, bq, Wk, bk, Wv, bv)
    nc = _get_nc()
    res = run_bass_kernel_spmd(nc, in_maps, core_ids=list(range(B)),
                               trace=_trace, tmpdir=_tmpdir)
    out = np.stack([res.results[i]["out"].reshape(C, H, W) for i in range(B)])
    if _trace:
        _cache["last_result"] = res
    return out.astype(np.float32)



# revision 2
# speedup vs baseline: 1.0100x; 1.0100x over previous
"""AreaAttention TRN2 kernel v3: fp16 QK + fp8-DoubleRow PV/denominator.

Math (per sample, C=128, N=4096):
    scores[m,n] = sum_c k[c,m] q[c,n];  k = x*colsum(Wk)+bk, q = Wq@x+bq
  bk adds a per-query constant to scores -> cancels in softmax. Folding
  wks=colsum(Wk) into Wq host-side (Wqw = wks[:,None]*Wq):
    scores_eff[m,n] = sum_c x[c,m] * qs[c,n],  qs = Wqw@x + bq*wks
  => no k tensor; the key-side QK operand is just x16.

Design notes (measured on HW):
  - fp16 matmul = 1 col/cycle @2.4GHz (216ns/512col). fp8 DoubleRow is only
    a real 2x when the contraction is genuinely 256: PV and the ones-rowsum
    contract over key pairs -> half the matmuls. The QK c-split [64,2,*]
    trick does NOT pay (64-partition shapes stream 1 col/cycle anyway), so
    QK stays fp16 (also better precision).
  - exp on ACT (exact exp -> fp8e5) + DVE (Schraudolph e5m2 u8 bits) at
    [128,512] grain through a 4-deep rotating PSUM sc pool.
  - denominator via ones8 DoubleRow rowsum on the PE (PSUM-accumulated per
    block) - no DVE chain adds at all.
  - GpSimd can't touch PSUM: it does SBUF residual adds + x16 DMA issue.
  - PSUM: pv[128,1024](2) + rs[128,1024](2) + 4x sc[128,512](4) = 8 banks.
  - PV/RS deferred one chunk so the in-order PE never stalls on fresh exps.
"""
import numpy as np
import ml_dtypes

C = 128
N = 4096
NB = 1024
NBLK = N // NB     # 4
MCH = N // C       # 32 m-chunks
NPAIR = MCH // 2   # 16
SCALE = 1.0 / np.sqrt(np.float32(C))
A5 = 4.0 / np.log(2.0)     # e5m2 Schraudolph slope
B5 = 59.75                 # e5m2 Schraudolph bias (HW cast rounds)

e4np = ml_dtypes.float8_e4m3

_cache = {}


def _build_nc():
    import concourse.tile as tile
    from concourse import bacc, mybir

    f32 = mybir.dt.float32
    f16 = mybir.dt.float16
    f8e4 = mybir.dt.float8e4
    f8e5 = mybir.dt.float8e5
    u8 = mybir.dt.uint8
    ADD = mybir.AluOpType.add
    MUL = mybir.AluOpType.mult
    EXP = mybir.ActivationFunctionType.Exp
    DR = mybir.MatmulPerfMode.DoubleRow

    nc = bacc.Bacc("TRN2", target_bir_lowering=False)

    x16_d = nc.dram_tensor("x16", [C, N], f16, kind="ExternalInput")
    # packed fp16 weights: [WqwT | WvT]
    w16_d = nc.dram_tensor("w16", [C, 2 * C], f16, kind="ExternalInput")
    smalls_d = nc.dram_tensor("smalls", [C, 2], f32, kind="ExternalInput")
    out_d = nc.dram_tensor("out", [C, N], f16, kind="ExternalOutput")

    DVE_MULT = float(A5 * SCALE)

    with tile.TileContext(nc) as tc:
        with tc.tile_pool(name="big", bufs=1) as big, \
             tc.tile_pool(name="small", bufs=1) as small, \
             tc.tile_pool(name="es_pool", bufs=8) as es_pool, \
             tc.tile_pool(name="work", bufs=2) as work, \
             tc.tile_pool(name="ps_sc", bufs=4, space="PSUM") as ps_sc, \
             tc.tile_pool(name="ps_pv", bufs=1, space="PSUM") as ps_pv, \
             tc.tile_pool(name="ps_rs", bufs=1, space="PSUM") as ps_rs:

            # x16 pieces: piece p covers n-cols [p*NB,(p+1)*NB) = key chunks
            # 8p..8p+7 (QK stationaries) + block p's residual + qproj moving.
            x16_t = [big.tile([C, NB], f16, tag=f"x16_{b}", name=f"x16_{b}")
                     for b in range(4)]
            q16_t = [big.tile([C, NB], f16, tag=f"q16_{b}", name=f"q16_{b}")
                     for b in range(4)]
            # v8 group g: v chunks 4g..4g+3 in [m, chunk, c] layout (fp8)
            v8_t = [big.tile([C, 4, C], f8e4, tag=f"v8_{g}", name=f"v8_{g}")
                    for g in range(8)]

            smalls = small.tile([C, 2], f32, tag="smalls")
            w16 = small.tile([C, 2 * C], f16, tag="w16")
            ones8 = small.tile([C, 2, C], f8e4, tag="ones8")
            ones16 = small.tile([C, C], f16, tag="ones16")
            wqwt16 = w16[:, 0:C]
            wvt16 = w16[:, C:2 * C]
            bqw = smalls[:, 0:1]
            bv16 = smalls[:, 1:2]

            nc.scalar.dma_start(smalls[:], smalls_d[:])
            nc.scalar.dma_start(w16[:], w16_d[:])
            for p in range(4):
                eng = nc.sync if p % 2 == 0 else nc.scalar
                eng.dma_start(x16_t[p][:], x16_d[:, p * NB:(p + 1) * NB])
            nc.vector.memset(ones8[:], 1.0)
            nc.vector.memset(ones16[:], 1.0)

            def x16_chunk(j):
                p, r = divmod(j * C, NB)
                return x16_t[p][:, r:r + C]

            def qproj(b, on_dve):
                """q16 for block b: 2 fp16 matmuls + bias-add -> fp16."""
                for h in range(2):
                    qp = ps_sc.tile([C, 512], f32, tag="sc", name=f"qp{b}_{h}")
                    nc.tensor.matmul(qp[:], wqwt16,
                                     x16_t[b][:, h * 512:(h + 1) * 512],
                                     start=True, stop=True)
                    dst = q16_t[b][:, h * 512:(h + 1) * 512]
                    if on_dve:
                        nc.vector.tensor_scalar(dst, qp[:], bqw, None, op0=ADD)
                    else:
                        nc.scalar.add(dst, qp[:], bqw)

            def vproj(g):
                """v chunks 4g..4g+3 (fp16 matmuls) -> v8 group g (fp8)."""
                vp = ps_sc.tile([C, 512], f32, tag="sc", name=f"vp{g}")
                for t in range(4):
                    nc.tensor.matmul(vp[:, t * C:(t + 1) * C],
                                     x16_chunk(4 * g + t), wvt16,
                                     start=True, stop=True)
                if g % 2 == 0:
                    nc.vector.tensor_scalar(v8_t[g][:], vp[:], bv16, None,
                                            op0=ADD)
                else:
                    nc.scalar.add(v8_t[g][:], vp[:], bv16)

            # exp engine per half-op: ACT exact exp vs DVE Schraudolph.
            ecnt = [0]

            def emit_exp(es_t, u, h, sc):
                dst = es_t[:, u, h * 512:(h + 1) * 512]
                on_act = (ecnt[0] * 5) % 9 < 5
                ecnt[0] += 1
                if on_act:
                    nc.scalar.activation(dst, sc[:], EXP, bias=0.0,
                                         scale=float(SCALE))
                else:
                    nc.vector.tensor_scalar(dst.bitcast(u8), sc[:],
                                            DVE_MULT, float(B5),
                                            op0=MUL, op1=ADD)

            def tail(b, pv, rs):
                n0 = b * NB
                last = b == NBLK - 1
                rb = work.tile([C, NB], f32, tag="rb", name=f"rb{b}")
                ep = work.tile([C, NB], f32, tag="ep", name=f"ep{b}")
                ost = work.tile([C, NB], f16, tag="ost", name=f"ost{b}")
                if last:
                    # exposed tail: quarter-grain DVE chain so each output
                    # quarter DMAs out while the next quarter computes
                    for q in range(4):
                        qsl = slice(q * 256, (q + 1) * 256)
                        nc.vector.reciprocal_approx_fast(out=rb[:, qsl],
                                                         in_=rs[:, qsl])
                        nc.vector.tensor_tensor(ep[:, qsl], pv[:, qsl],
                                                rb[:, qsl], op=MUL)
                        nc.vector.tensor_tensor(ost[:, qsl], ep[:, qsl],
                                                x16_t[b][:, qsl], op=ADD)
                        eng = nc.sync if q % 2 == 0 else nc.scalar
                        eng.dma_start(out_d[:, n0 + qsl.start:n0 + qsl.stop],
                                      ost[:, qsl])
                    return
                for h in range(2):
                    hsl = slice(h * 512, (h + 1) * 512)
                    nc.vector.reciprocal_approx_fast(out=rb[:, hsl],
                                                     in_=rs[:, hsl])
                    nc.vector.tensor_tensor(ep[:, hsl], pv[:, hsl], rb[:, hsl],
                                            op=MUL)
                    nc.gpsimd.tensor_tensor(ost[:, hsl], ep[:, hsl],
                                            x16_t[b][:, hsl], op=ADD)
                    eng = nc.sync if h == 0 else nc.scalar
                    eng.dma_start(out_d[:, n0 + hsl.start:n0 + hsl.stop],
                                  ost[:, hsl])

            qproj(0, on_dve=True)

            for b in range(NBLK):
                pv = ps_pv.tile([C, NB], f32, tag="pv", name=f"pv{b}")
                rs = ps_rs.tile([C, NB], f32, tag="rs", name=f"rs{b}")

                def flush_pv(pend):
                    # both PV halves share one v8-pair LDWEIGHTS, both RS
                    # halves share the ones8 load: 2 weight loads per pair
                    # instead of 4 (each reload stalls the PE ~160ns).
                    jp, et = pend
                    for h in range(2):
                        hsl = slice(h * 512, (h + 1) * 512)
                        nc.tensor.matmul(pv[:, hsl],
                                         v8_t[jp // 2][:, (jp % 2) * 2:(jp % 2) * 2 + 2, :],
                                         et[:, :, hsl],
                                         start=(jp == 0), stop=(jp == NPAIR - 1),
                                         perf_mode=DR)
                    for h in range(2):
                        hsl = slice(h * 512, (h + 1) * 512)
                        nc.tensor.matmul(rs[:, hsl], ones8[:],
                                         et[:, :, hsl],
                                         start=(jp == 0), stop=(jp == NPAIR - 1),
                                         perf_mode=DR)

                es_t = None
                pend = None   # PV/RS deferred one chunk: the in-order PE
                # always has fresh QK work before possibly-exp-gated reads
                for j in range(MCH):
                    if b == 0 and j % 4 == 0:
                        vproj(j // 4)
                    if j % 2 == 0:
                        es_t = es_pool.tile([C, 2, NB], f8e5, tag="es",
                                            name=f"es{b}_{j // 2}")
                    for h in range(2):
                        sc = ps_sc.tile([C, 512], f32, tag="sc",
                                        name=f"sc{b}_{j}_{h}")
                        nc.tensor.matmul(
                            sc[:], x16_chunk(j),
                            q16_t[b][:, h * 512:(h + 1) * 512],
                            start=True, stop=True)
                        emit_exp(es_t, j % 2, h, sc)
                    if pend is not None:
                        flush_pv(pend)
                        pend = None
                    if j % 2 == 1:
                        pend = (j // 2, es_t)
                    if j == 19 and b < NBLK - 1:
                        qproj(b + 1, on_dve=(b % 2 == 0))
                flush_pv(pend)
                tail(b, pv, rs)

    nc.finalize()
    return nc


def _get_nc():
    if "nc" not in _cache:
        _cache["nc"] = _build_nc()
    return _cache["nc"]


def make_in_maps(x, Wq, bq, Wk, bk, Wv, bv):
    x = np.asarray(x, dtype=np.float32)
    B = x.shape[0]
    wks = np.asarray(Wk, np.float32).sum(axis=0)            # [C]
    Wqw = np.asarray(Wq, np.float32) * wks[:, None]
    bqw = np.asarray(bq, np.float32) * wks
    w16 = np.ascontiguousarray(np.concatenate(
        [Wqw.T.astype(np.float16), np.asarray(Wv, np.float32).T.astype(np.float16)],
        axis=1))
    smalls = np.ascontiguousarray(
        np.stack([bqw, np.asarray(bv, np.float32)], axis=1).astype(np.float32))

    in_maps = []
    for i in range(B):
        xf = np.ascontiguousarray(x[i].reshape(C, N))
        in_maps.append({
            "x16": xf.astype(np.float16),
            "w16": w16, "smalls": smalls,
        })
    return in_maps


def kernel(x, Wq, bq, Wk, bk, Wv, bv, _trace=False, _tmpdir=None):
    from concourse.bass_utils import run_bass_kernel_spmd

    x = np.asarray(x, dtype=np.float32)
    B, c, H, W = x.shape
    assert (c, H * W) == (C, N), (c, H, W)
    in_maps = make_in_maps(x, Wq, bq, Wk, bk, Wv, bv)
    nc = _get_nc()
    res = run_bass_kernel_spmd(nc, in_maps, core_ids=list(range(B)),
                               trace=_trace, tmpdir=_tmpdir)
    out = np.stack([
        np.asarray(res.results[i]["out"]).astype(np.float32).reshape(C, H, W)
        for i in range(B)
    ])
    if _trace:
        _cache["last_result"] = res
    return out


# revision 3
# speedup vs baseline: 1.0168x; 1.0068x over previous
"""AreaAttention TRN2 kernel v3: fp16 QK + fp8-DoubleRow PV/denominator.

Math (per sample, C=128, N=4096):
    scores[m,n] = sum_c k[c,m] q[c,n];  k = x*colsum(Wk)+bk, q = Wq@x+bq
  bk adds a per-query constant to scores -> cancels in softmax. Folding
  wks=colsum(Wk) into Wq host-side (Wqw = wks[:,None]*Wq):
    scores_eff[m,n] = sum_c x[c,m] * qs[c,n],  qs = Wqw@x + bq*wks
  => no k tensor; the key-side QK operand is just x16.

Design notes (measured on HW):
  - fp16 matmul = 1 col/cycle @2.4GHz (216ns/512col). fp8 DoubleRow is only
    a real 2x when the contraction is genuinely 256: PV and the ones-rowsum
    contract over key pairs -> half the matmuls. The QK c-split [64,2,*]
    trick does NOT pay (64-partition shapes stream 1 col/cycle anyway), so
    QK stays fp16 (also better precision).
  - exp on ACT (exact exp -> fp8e5) + DVE (Schraudolph e5m2 u8 bits) at
    [128,512] grain through a 4-deep rotating PSUM sc pool.
  - denominator via ones8 DoubleRow rowsum on the PE (PSUM-accumulated per
    block) - no DVE chain adds at all.
  - GpSimd can't touch PSUM: it does SBUF residual adds + x16 DMA issue.
  - PSUM: pv[128,1024](2) + rs[128,1024](2) + 4x sc[128,512](4) = 8 banks.
  - PV/RS deferred one chunk so the in-order PE never stalls on fresh exps.
"""
import numpy as np
import ml_dtypes

C = 128
N = 4096
NB = 1024
NBLK = N // NB     # 4
MCH = N // C       # 32 m-chunks
NPAIR = MCH // 2   # 16
SCALE = 1.0 / np.sqrt(np.float32(C))
A5 = 4.0 / np.log(2.0)     # e5m2 Schraudolph slope
B5 = 59.75                 # e5m2 Schraudolph bias (HW cast rounds)

e4np = ml_dtypes.float8_e4m3

_cache = {}


def _build_nc():
    import concourse.tile as tile
    from concourse import bacc, mybir

    f32 = mybir.dt.float32
    f16 = mybir.dt.float16
    f8e4 = mybir.dt.float8e4
    f8e5 = mybir.dt.float8e5
    u8 = mybir.dt.uint8
    ADD = mybir.AluOpType.add
    MUL = mybir.AluOpType.mult
    EXP = mybir.ActivationFunctionType.Exp
    DR = mybir.MatmulPerfMode.DoubleRow

    nc = bacc.Bacc("TRN2", target_bir_lowering=False)

    x16_d = nc.dram_tensor("x16", [C, N], f16, kind="ExternalInput")
    # packed fp16 weights: [WqwT | WvT]
    w16_d = nc.dram_tensor("w16", [C, 2 * C], f16, kind="ExternalInput")
    smalls_d = nc.dram_tensor("smalls", [C, 2], f32, kind="ExternalInput")
    out_d = nc.dram_tensor("out", [C, N], f16, kind="ExternalOutput")

    DVE_MULT = float(A5 * SCALE)

    with tile.TileContext(nc) as tc:
        with tc.tile_pool(name="big", bufs=1) as big, \
             tc.tile_pool(name="small", bufs=1) as small, \
             tc.tile_pool(name="es_pool", bufs=8) as es_pool, \
             tc.tile_pool(name="work", bufs=2) as work, \
             tc.tile_pool(name="ps_sc", bufs=4, space="PSUM") as ps_sc, \
             tc.tile_pool(name="ps_pv", bufs=1, space="PSUM") as ps_pv, \
             tc.tile_pool(name="ps_rs", bufs=1, space="PSUM") as ps_rs:

            # x16 pieces: piece p covers n-cols [p*NB,(p+1)*NB) = key chunks
            # 8p..8p+7 (QK stationaries) + block p's residual + qproj moving.
            x16_t = [big.tile([C, NB], f16, tag=f"x16_{b}", name=f"x16_{b}")
                     for b in range(4)]
            q16_t = [big.tile([C, NB], f16, tag=f"q16_{b}", name=f"q16_{b}")
                     for b in range(4)]
            # v8 group g: v chunks 4g..4g+3 in [m, chunk, c] layout (fp8)
            v8_t = [big.tile([C, 4, C], f8e4, tag=f"v8_{g}", name=f"v8_{g}")
                    for g in range(8)]

            smalls = small.tile([C, 2], f32, tag="smalls")
            w16 = small.tile([C, 2 * C], f16, tag="w16")
            ones8 = small.tile([C, 2, C], f8e4, tag="ones8")
            ones16 = small.tile([C, C], f16, tag="ones16")
            wqwt16 = w16[:, 0:C]
            wvt16 = w16[:, C:2 * C]
            bqw = smalls[:, 0:1]
            bv16 = smalls[:, 1:2]

            nc.scalar.dma_start(smalls[:], smalls_d[:])
            nc.scalar.dma_start(w16[:], w16_d[:])
            for p in range(4):
                eng = nc.sync if p % 2 == 0 else nc.scalar
                eng.dma_start(x16_t[p][:], x16_d[:, p * NB:(p + 1) * NB])
            nc.vector.memset(ones8[:], 1.0)
            nc.vector.memset(ones16[:], 1.0)

            def x16_chunk(j):
                p, r = divmod(j * C, NB)
                return x16_t[p][:, r:r + C]

            def qproj(b, on_dve):
                """q16 for block b: 2 fp16 matmuls + bias-add -> fp16."""
                for h in range(2):
                    qp = ps_sc.tile([C, 512], f32, tag="sc", name=f"qp{b}_{h}")
                    nc.tensor.matmul(qp[:], wqwt16,
                                     x16_t[b][:, h * 512:(h + 1) * 512],
                                     start=True, stop=True)
                    dst = q16_t[b][:, h * 512:(h + 1) * 512]
                    if on_dve:
                        nc.vector.tensor_scalar(dst, qp[:], bqw, None, op0=ADD)
                    else:
                        nc.scalar.add(dst, qp[:], bqw)

            def vproj(g):
                """v chunks 4g..4g+3 (fp16 matmuls) -> v8 group g (fp8)."""
                vp = ps_sc.tile([C, 512], f32, tag="sc", name=f"vp{g}")
                for t in range(4):
                    nc.tensor.matmul(vp[:, t * C:(t + 1) * C],
                                     x16_chunk(4 * g + t), wvt16,
                                     start=True, stop=True)
                if g % 2 == 0:
                    nc.vector.tensor_scalar(v8_t[g][:], vp[:], bv16, None,
                                            op0=ADD)
                else:
                    nc.scalar.add(v8_t[g][:], vp[:], bv16)

            # exp engine per half-op: ACT exact exp vs DVE Schraudolph.
            ecnt = [0]

            def emit_exp(es_t, u, h, sc):
                dst = es_t[:, u, h * 512:(h + 1) * 512]
                on_act = (ecnt[0] * 5) % 9 < 5
                ecnt[0] += 1
                if on_act:
                    nc.scalar.activation(dst, sc[:], EXP, bias=0.0,
                                         scale=float(SCALE))
                else:
                    nc.vector.tensor_scalar(dst.bitcast(u8), sc[:],
                                            DVE_MULT, float(B5),
                                            op0=MUL, op1=ADD)

            def tail(b, pv, rs):
                n0 = b * NB
                last = b == NBLK - 1
                rb = work.tile([C, NB], f32, tag="rb", name=f"rb{b}")
                ep = work.tile([C, NB], f32, tag="ep", name=f"ep{b}")
                ost = work.tile([C, NB], f16, tag="ost", name=f"ost{b}")
                if last:
                    # exposed tail: quarter-grain DVE chain so each output
                    # quarter DMAs out while the next quarter computes
                    for q in range(4):
                        qsl = slice(q * 256, (q + 1) * 256)
                        nc.vector.reciprocal_approx_fast(out=rb[:, qsl],
                                                         in_=rs[:, qsl])
                        nc.vector.tensor_tensor(ep[:, qsl], pv[:, qsl],
                                                rb[:, qsl], op=MUL)
                        nc.vector.tensor_tensor(ost[:, qsl], ep[:, qsl],
                                                x16_t[b][:, qsl], op=ADD)
                        eng = nc.sync if q % 2 == 0 else nc.scalar
                        eng.dma_start(out_d[:, n0 + qsl.start:n0 + qsl.stop],
                                      ost[:, qsl])
                    return
                for h in range(2):
                    hsl = slice(h * 512, (h + 1) * 512)
                    nc.vector.reciprocal_approx_fast(out=rb[:, hsl],
                                                     in_=rs[:, hsl])
                    nc.vector.tensor_tensor(ep[:, hsl], pv[:, hsl], rb[:, hsl],
                                            op=MUL)
                    nc.gpsimd.tensor_tensor(ost[:, hsl], ep[:, hsl],
                                            x16_t[b][:, hsl], op=ADD)
                    eng = nc.sync if h == 0 else nc.scalar
                    eng.dma_start(out_d[:, n0 + hsl.start:n0 + hsl.stop],
                                  ost[:, hsl])

            qproj(0, on_dve=True)

            for b in range(NBLK):
                pv = ps_pv.tile([C, NB], f32, tag="pv", name=f"pv{b}")
                rs = ps_rs.tile([C, NB], f32, tag="rs", name=f"rs{b}")

                def flush_pv(pend):
                    # both PV halves share one v8-pair LDWEIGHTS, both RS
                    # halves share the ones8 load: 2 weight loads per pair
                    # instead of 4 (each reload stalls the PE ~160ns).
                    jp, et = pend
                    for h in range(2):
                        hsl = slice(h * 512, (h + 1) * 512)
                        nc.tensor.matmul(pv[:, hsl],
                                         v8_t[jp // 2][:, (jp % 2) * 2:(jp % 2) * 2 + 2, :],
                                         et[:, :, hsl],
                                         start=(jp == 0), stop=(jp == NPAIR - 1),
                                         perf_mode=DR)
                    for h in range(2):
                        hsl = slice(h * 512, (h + 1) * 512)
                        nc.tensor.matmul(rs[:, hsl], ones8[:],
                                         et[:, :, hsl],
                                         start=(jp == 0), stop=(jp == NPAIR - 1),
                                         perf_mode=DR)

                es_t = None
                pendq = []   # PV/RS deferred and flushed TWO pairs at a
                # time: fewer fp16<->fp8 mode transitions in the PE stream
                # (each first-matmul-after-transition costs ~150ns), and the
                # in-order PE always has fresh QK work before exp-gated reads
                for j in range(MCH):
                    if b == 0 and j % 4 == 0:
                        vproj(j // 4)
                    if j % 2 == 0:
                        es_t = es_pool.tile([C, 2, NB], f8e5, tag="es",
                                            name=f"es{b}_{j // 2}")
                    for h in range(2):
                        sc = ps_sc.tile([C, 512], f32, tag="sc",
                                        name=f"sc{b}_{j}_{h}")
                        nc.tensor.matmul(
                            sc[:], x16_chunk(j),
                            q16_t[b][:, h * 512:(h + 1) * 512],
                            start=True, stop=True)
                        emit_exp(es_t, j % 2, h, sc)
                    if j % 2 == 1:
                        pendq.append((j // 2, es_t))
                        if len(pendq) == 2:
                            for pend in pendq:
                                flush_pv(pend)
                            pendq = []
                    if j == 19 and b < NBLK - 1:
                        qproj(b + 1, on_dve=(b % 2 == 0))
                for pend in pendq:
                    flush_pv(pend)
                tail(b, pv, rs)

    nc.finalize()
    return nc


def _get_nc():
    if "nc" not in _cache:
        _cache["nc"] = _build_nc()
    return _cache["nc"]


def make_in_maps(x, Wq, bq, Wk, bk, Wv, bv):
    x = np.asarray(x, dtype=np.float32)
    B = x.shape[0]
    wks = np.asarray(Wk, np.float32).sum(axis=0)            # [C]
    Wqw = np.asarray(Wq, np.float32) * wks[:, None]
    bqw = np.asarray(bq, np.float32) * wks
    w16 = np.ascontiguousarray(np.concatenate(
        [Wqw.T.astype(np.float16), np.asarray(Wv, np.float32).T.astype(np.float16)],
        axis=1))
    smalls = np.ascontiguousarray(
        np.stack([bqw, np.asarray(bv, np.float32)], axis=1).astype(np.float32))

    in_maps = []
    for i in range(B):
        xf = np.ascontiguousarray(x[i].reshape(C, N))
        in_maps.append({
            "x16": xf.astype(np.float16),
            "w16": w16, "smalls": smalls,
        })
    return in_maps


def kernel(x, Wq, bq, Wk, bk, Wv, bv, _trace=False, _tmpdir=None):
    from concourse.bass_utils import run_bass_kernel_spmd

    x = np.asarray(x, dtype=np.float32)
    B, c, H, W = x.shape
    assert (c, H * W) == (C, N), (c, H, W)
    in_maps = make_in_maps(x, Wq, bq, Wk, bk, Wv, bv)
    nc = _get_nc()
    res = run_bass_kernel_spmd(nc, in_maps, core_ids=list(range(B)),
                               trace=_trace, tmpdir=_tmpdir)
    out = np.stack([
        np.asarray(res.results[i]["out"]).astype(np.float32).reshape(C, H, W)
        for i in range(B)
    ])
    if _trace:
        _cache["last_result"] = res
    return out


# revision 4
# speedup vs baseline: 1.0252x; 1.0082x over previous
"""AreaAttention TRN2 kernel v3: fp16 QK + fp8-DoubleRow PV/denominator.

Math (per sample, C=128, N=4096):
    scores[m,n] = sum_c k[c,m] q[c,n];  k = x*colsum(Wk)+bk, q = Wq@x+bq
  bk adds a per-query constant to scores -> cancels in softmax. Folding
  wks=colsum(Wk) into Wq host-side (Wqw = wks[:,None]*Wq):
    scores_eff[m,n] = sum_c x[c,m] * qs[c,n],  qs = Wqw@x + bq*wks
  => no k tensor; the key-side QK operand is just x16.

Design notes (measured on HW):
  - fp16 matmul = 1 col/cycle @2.4GHz (216ns/512col). fp8 DoubleRow is only
    a real 2x when the contraction is genuinely 256: PV and the ones-rowsum
    contract over key pairs -> half the matmuls. The QK c-split [64,2,*]
    trick does NOT pay (64-partition shapes stream 1 col/cycle anyway), so
    QK stays fp16 (also better precision).
  - exp on ACT (exact exp -> fp8e5) + DVE (Schraudolph e5m2 u8 bits) at
    [128,512] grain through a 4-deep rotating PSUM sc pool.
  - denominator via ones8 DoubleRow rowsum on the PE (PSUM-accumulated per
    block) - no DVE chain adds at all.
  - GpSimd can't touch PSUM: it does SBUF residual adds + x16 DMA issue.
  - PSUM: pv[128,1024](2) + rs[128,1024](2) + 4x sc[128,512](4) = 8 banks.
  - PV/RS deferred one chunk so the in-order PE never stalls on fresh exps.
"""
import numpy as np
import ml_dtypes

C = 128
N = 4096
NB = 1024
NBLK = N // NB     # 4
MCH = N // C       # 32 m-chunks
NPAIR = MCH // 2   # 16
SCALE = 1.0 / np.sqrt(np.float32(C))
A5 = 4.0 / np.log(2.0)     # e5m2 Schraudolph slope
B5 = 59.75                 # e5m2 Schraudolph bias (HW cast rounds)

e4np = ml_dtypes.float8_e4m3

_cache = {}


def _build_nc():
    import concourse.tile as tile
    from concourse import bacc, mybir

    f32 = mybir.dt.float32
    f16 = mybir.dt.float16
    f8e4 = mybir.dt.float8e4
    f8e5 = mybir.dt.float8e5
    u8 = mybir.dt.uint8
    ADD = mybir.AluOpType.add
    MUL = mybir.AluOpType.mult
    EXP = mybir.ActivationFunctionType.Exp
    DR = mybir.MatmulPerfMode.DoubleRow

    nc = bacc.Bacc("TRN2", target_bir_lowering=False)

    x16_d = nc.dram_tensor("x16", [C, N], f16, kind="ExternalInput")
    # packed fp16 weights: [WqwT | WvT]
    w16_d = nc.dram_tensor("w16", [C, 2 * C], f16, kind="ExternalInput")
    smalls_d = nc.dram_tensor("smalls", [C, 2], f32, kind="ExternalInput")
    out_d = nc.dram_tensor("out", [C, N], f16, kind="ExternalOutput")

    DVE_MULT = float(A5 * SCALE)

    with tile.TileContext(nc) as tc:
        with tc.tile_pool(name="big", bufs=1) as big, \
             tc.tile_pool(name="small", bufs=1) as small, \
             tc.tile_pool(name="es_pool", bufs=8) as es_pool, \
             tc.tile_pool(name="work", bufs=2) as work, \
             tc.tile_pool(name="ps_sc", bufs=4, space="PSUM") as ps_sc, \
             tc.tile_pool(name="ps_pv", bufs=1, space="PSUM") as ps_pv, \
             tc.tile_pool(name="ps_rs", bufs=1, space="PSUM") as ps_rs:

            # x16 pieces: piece p covers n-cols [p*NB,(p+1)*NB) = key chunks
            # 8p..8p+7 (QK stationaries) + block p's residual + qproj moving.
            x16_t = [big.tile([C, NB], f16, tag=f"x16_{b}", name=f"x16_{b}")
                     for b in range(4)]
            q16_t = [big.tile([C, NB], f16, tag=f"q16_{b}", name=f"q16_{b}")
                     for b in range(4)]
            # v8 group g: v chunks 4g..4g+3 in [m, chunk, c] layout (fp8)
            v8_t = [big.tile([C, 4, C], f8e4, tag=f"v8_{g}", name=f"v8_{g}")
                    for g in range(8)]

            smalls = small.tile([C, 2], f32, tag="smalls")
            w16 = small.tile([C, 2 * C], f16, tag="w16")
            ones8 = small.tile([C, 2, C], f8e4, tag="ones8")
            ones16 = small.tile([C, C], f16, tag="ones16")
            wqwt16 = w16[:, 0:C]
            wvt16 = w16[:, C:2 * C]
            bqw = smalls[:, 0:1]
            bv16 = smalls[:, 1:2]

            nc.scalar.dma_start(smalls[:], smalls_d[:])
            nc.scalar.dma_start(w16[:], w16_d[:])
            # piece 0 gates qproj(0) -> first matmul: split it across the
            # sync + (idle) swdge queues so it lands ~1us earlier
            nc.sync.dma_start(x16_t[0][:, 0:512], x16_d[:, 0:512])
            nc.gpsimd.dma_start(x16_t[0][:, 512:NB], x16_d[:, 512:NB])
            for p in range(1, 4):
                eng = nc.sync if p % 2 == 0 else nc.scalar
                eng.dma_start(x16_t[p][:], x16_d[:, p * NB:(p + 1) * NB])
            nc.vector.memset(ones8[:], 1.0)
            nc.vector.memset(ones16[:], 1.0)

            def x16_chunk(j):
                p, r = divmod(j * C, NB)
                return x16_t[p][:, r:r + C]

            def qproj(b, on_dve):
                """q16 for block b: 2 fp16 matmuls + bias-add -> fp16."""
                for h in range(2):
                    qp = ps_sc.tile([C, 512], f32, tag="sc", name=f"qp{b}_{h}")
                    nc.tensor.matmul(qp[:], wqwt16,
                                     x16_t[b][:, h * 512:(h + 1) * 512],
                                     start=True, stop=True)
                    dst = q16_t[b][:, h * 512:(h + 1) * 512]
                    if on_dve:
                        nc.vector.tensor_scalar(dst, qp[:], bqw, None, op0=ADD)
                    else:
                        nc.scalar.add(dst, qp[:], bqw)

            def vproj(g):
                """v chunks 4g..4g+3 (fp16 matmuls) -> v8 group g (fp8)."""
                vp = ps_sc.tile([C, 512], f32, tag="sc", name=f"vp{g}")
                for t in range(4):
                    nc.tensor.matmul(vp[:, t * C:(t + 1) * C],
                                     x16_chunk(4 * g + t), wvt16,
                                     start=True, stop=True)
                if g % 2 == 0:
                    nc.vector.tensor_scalar(v8_t[g][:], vp[:], bv16, None,
                                            op0=ADD)
                else:
                    nc.scalar.add(v8_t[g][:], vp[:], bv16)

            # exp engine per half-op: ACT exact exp vs DVE Schraudolph.
            ecnt = [0]

            def emit_exp(es_t, u, h, sc):
                dst = es_t[:, u, h * 512:(h + 1) * 512]
                on_act = (ecnt[0] * 5) % 9 < 5
                ecnt[0] += 1
                if on_act:
                    nc.scalar.activation(dst, sc[:], EXP, bias=0.0,
                                         scale=float(SCALE))
                else:
                    nc.vector.tensor_scalar(dst.bitcast(u8), sc[:],
                                            DVE_MULT, float(B5),
                                            op0=MUL, op1=ADD)

            def tail(b, pv, rs):
                n0 = b * NB
                last = b == NBLK - 1
                rb = work.tile([C, NB], f32, tag="rb", name=f"rb{b}")
                ep = work.tile([C, NB], f32, tag="ep", name=f"ep{b}")
                ost = work.tile([C, NB], f16, tag="ost", name=f"ost{b}")
                if last:
                    # exposed tail: quarter-grain DVE chain so each output
                    # quarter DMAs out while the next quarter computes
                    for q in range(4):
                        qsl = slice(q * 256, (q + 1) * 256)
                        nc.vector.reciprocal_approx_fast(out=rb[:, qsl],
                                                         in_=rs[:, qsl])
                        nc.vector.tensor_tensor(ep[:, qsl], pv[:, qsl],
                                                rb[:, qsl], op=MUL)
                        nc.vector.tensor_tensor(ost[:, qsl], ep[:, qsl],
                                                x16_t[b][:, qsl], op=ADD)
                        eng = nc.sync if q % 2 == 0 else nc.scalar
                        eng.dma_start(out_d[:, n0 + qsl.start:n0 + qsl.stop],
                                      ost[:, qsl])
                    return
                for h in range(2):
                    hsl = slice(h * 512, (h + 1) * 512)
                    nc.vector.reciprocal_approx_fast(out=rb[:, hsl],
                                                     in_=rs[:, hsl])
                    nc.vector.tensor_tensor(ep[:, hsl], pv[:, hsl], rb[:, hsl],
                                            op=MUL)
                    nc.gpsimd.tensor_tensor(ost[:, hsl], ep[:, hsl],
                                            x16_t[b][:, hsl], op=ADD)
                    eng = nc.sync if h == 0 else nc.scalar
                    eng.dma_start(out_d[:, n0 + hsl.start:n0 + hsl.stop],
                                  ost[:, hsl])

            qproj(0, on_dve=True)

            for b in range(NBLK):
                pv = ps_pv.tile([C, NB], f32, tag="pv", name=f"pv{b}")
                rs = ps_rs.tile([C, NB], f32, tag="rs", name=f"rs{b}")

                def flush_pv(pend):
                    # both PV halves share one v8-pair LDWEIGHTS, both RS
                    # halves share the ones8 load: 2 weight loads per pair
                    # instead of 4 (each reload stalls the PE ~160ns).
                    jp, et = pend
                    for h in range(2):
                        hsl = slice(h * 512, (h + 1) * 512)
                        nc.tensor.matmul(pv[:, hsl],
                                         v8_t[jp // 2][:, (jp % 2) * 2:(jp % 2) * 2 + 2, :],
                                         et[:, :, hsl],
                                         start=(jp == 0), stop=(jp == NPAIR - 1),
                                         perf_mode=DR)
                    for h in range(2):
                        hsl = slice(h * 512, (h + 1) * 512)
                        nc.tensor.matmul(rs[:, hsl], ones8[:],
                                         et[:, :, hsl],
                                         start=(jp == 0), stop=(jp == NPAIR - 1),
                                         perf_mode=DR)

                es_t = None
                pendq = []   # PV/RS deferred and flushed TWO pairs at a
                # time: fewer fp16<->fp8 mode transitions in the PE stream
                # (each first-matmul-after-transition costs ~150ns), and the
                # in-order PE always has fresh QK work before exp-gated reads
                for j in range(MCH):
                    if b == 0 and j % 4 == 0:
                        vproj(j // 4)
                    if j % 2 == 0:
                        es_t = es_pool.tile([C, 2, NB], f8e5, tag="es",
                                            name=f"es{b}_{j // 2}")
                    for h in range(2):
                        sc = ps_sc.tile([C, 512], f32, tag="sc",
                                        name=f"sc{b}_{j}_{h}")
                        nc.tensor.matmul(
                            sc[:], x16_chunk(j),
                            q16_t[b][:, h * 512:(h + 1) * 512],
                            start=True, stop=True)
                        emit_exp(es_t, j % 2, h, sc)
                    if j % 2 == 1:
                        pendq.append((j // 2, es_t))
                        if len(pendq) == 2:
                            for pend in pendq:
                                flush_pv(pend)
                            pendq = []
                    if j == 19 and b < NBLK - 1:
                        qproj(b + 1, on_dve=(b % 2 == 0))
                for pend in pendq:
                    flush_pv(pend)
                tail(b, pv, rs)

    nc.finalize()
    return nc


def _get_nc():
    if "nc" not in _cache:
        _cache["nc"] = _build_nc()
    return _cache["nc"]


def make_in_maps(x, Wq, bq, Wk, bk, Wv, bv):
    x = np.asarray(x, dtype=np.float32)
    B = x.shape[0]
    wks = np.asarray(Wk, np.float32).sum(axis=0)            # [C]
    Wqw = np.asarray(Wq, np.float32) * wks[:, None]
    bqw = np.asarray(bq, np.float32) * wks
    w16 = np.ascontiguousarray(np.concatenate(
        [Wqw.T.astype(np.float16), np.asarray(Wv, np.float32).T.astype(np.float16)],
        axis=1))
    smalls = np.ascontiguousarray(
        np.stack([bqw, np.asarray(bv, np.float32)], axis=1).astype(np.float32))

    in_maps = []
    for i in range(B):
        xf = np.ascontiguousarray(x[i].reshape(C, N))
        in_maps.append({
            "x16": xf.astype(np.float16),
            "w16": w16, "smalls": smalls,
        })
    return in_maps


def kernel(x, Wq, bq, Wk, bk, Wv, bv, _trace=False, _tmpdir=None):
    from concourse.bass_utils import run_bass_kernel_spmd

    x = np.asarray(x, dtype=np.float32)
    B, c, H, W = x.shape
    assert (c, H * W) == (C, N), (c, H, W)
    in_maps = make_in_maps(x, Wq, bq, Wk, bk, Wv, bv)
    nc = _get_nc()
    res = run_bass_kernel_spmd(nc, in_maps, core_ids=list(range(B)),
                               trace=_trace, tmpdir=_tmpdir)
    out = np.stack([
        np.asarray(res.results[i]["out"]).astype(np.float32).reshape(C, H, W)
        for i in range(B)
    ])
    if _trace:
        _cache["last_result"] = res
    return out


# revision 5
# speedup vs baseline: 1.0311x; 1.0058x over previous
"""AreaAttention TRN2 kernel v3: fp16 QK + fp8-DoubleRow PV/denominator.

Math (per sample, C=128, N=4096):
    scores[m,n] = sum_c k[c,m] q[c,n];  k = x*colsum(Wk)+bk, q = Wq@x+bq
  bk adds a per-query constant to scores -> cancels in softmax. Folding
  wks=colsum(Wk) into Wq host-side (Wqw = wks[:,None]*Wq):
    scores_eff[m,n] = sum_c x[c,m] * qs[c,n],  qs = Wqw@x + bq*wks
  => no k tensor; the key-side QK operand is just x16.

Design notes (measured on HW):
  - fp16 matmul = 1 col/cycle @2.4GHz (216ns/512col). fp8 DoubleRow is only
    a real 2x when the contraction is genuinely 256: PV and the ones-rowsum
    contract over key pairs -> half the matmuls. The QK c-split [64,2,*]
    trick does NOT pay (64-partition shapes stream 1 col/cycle anyway), so
    QK stays fp16 (also better precision).
  - exp on ACT (exact exp -> fp8e5) + DVE (Schraudolph e5m2 u8 bits) at
    [128,512] grain through a 4-deep rotating PSUM sc pool.
  - denominator via ones8 DoubleRow rowsum on the PE (PSUM-accumulated per
    block) - no DVE chain adds at all.
  - GpSimd can't touch PSUM: it does SBUF residual adds + x16 DMA issue.
  - PSUM: pv[128,1024](2) + rs[128,1024](2) + 4x sc[128,512](4) = 8 banks.
  - PV/RS deferred one chunk so the in-order PE never stalls on fresh exps.
"""
import numpy as np
import ml_dtypes

C = 128
N = 4096
NB = 1024
NBLK = N // NB     # 4
MCH = N // C       # 32 m-chunks
NPAIR = MCH // 2   # 16
SCALE = 1.0 / np.sqrt(np.float32(C))
A5 = 4.0 / np.log(2.0)     # e5m2 Schraudolph slope
B5 = 59.75                 # e5m2 Schraudolph bias (HW cast rounds)

e4np = ml_dtypes.float8_e4m3

_cache = {}


def _build_nc():
    import concourse.tile as tile
    from concourse import bacc, mybir

    f32 = mybir.dt.float32
    f16 = mybir.dt.float16
    f8e4 = mybir.dt.float8e4
    f8e5 = mybir.dt.float8e5
    u8 = mybir.dt.uint8
    ADD = mybir.AluOpType.add
    MUL = mybir.AluOpType.mult
    EXP = mybir.ActivationFunctionType.Exp
    DR = mybir.MatmulPerfMode.DoubleRow

    nc = bacc.Bacc("TRN2", target_bir_lowering=False)

    x16_d = nc.dram_tensor("x16", [C, N], f16, kind="ExternalInput")
    # packed fp16 weights: [WqwT | WvT]
    w16_d = nc.dram_tensor("w16", [C, 2 * C], f16, kind="ExternalInput")
    smalls_d = nc.dram_tensor("smalls", [C, 2], f32, kind="ExternalInput")
    out_d = nc.dram_tensor("out", [C, N], f16, kind="ExternalOutput")

    DVE_MULT = float(A5 * SCALE)

    with tile.TileContext(nc) as tc:
        with tc.tile_pool(name="big", bufs=1) as big, \
             tc.tile_pool(name="small", bufs=1) as small, \
             tc.tile_pool(name="es_pool", bufs=8) as es_pool, \
             tc.tile_pool(name="work", bufs=2) as work, \
             tc.tile_pool(name="ps_sc", bufs=4, space="PSUM") as ps_sc, \
             tc.tile_pool(name="ps_pv", bufs=1, space="PSUM") as ps_pv, \
             tc.tile_pool(name="ps_rs", bufs=1, space="PSUM") as ps_rs:

            # x16 pieces: piece p covers n-cols [p*NB,(p+1)*NB) = key chunks
            # 8p..8p+7 (QK stationaries) + block p's residual + qproj moving.
            x16_t = [big.tile([C, NB], f16, tag=f"x16_{b}", name=f"x16_{b}")
                     for b in range(4)]
            q16_t = [big.tile([C, NB], f16, tag=f"q16_{b}", name=f"q16_{b}")
                     for b in range(4)]
            # v8 group g: v chunks 4g..4g+3 in [m, chunk, c] layout (fp8)
            v8_t = [big.tile([C, 4, C], f8e4, tag=f"v8_{g}", name=f"v8_{g}")
                    for g in range(8)]

            smalls = small.tile([C, 2], f32, tag="smalls")
            w16 = small.tile([C, 2 * C], f16, tag="w16")
            ones8 = small.tile([C, 2, C], f8e4, tag="ones8")
            ones16 = small.tile([C, C], f16, tag="ones16")
            wqwt16 = w16[:, 0:C]
            wvt16 = w16[:, C:2 * C]
            bqw = smalls[:, 0:1]
            bv16 = smalls[:, 1:2]

            nc.scalar.dma_start(w16[:], w16_d[:])
            nc.scalar.dma_start(smalls[:], smalls_d[:])
            # piece 0 gates qproj(0) -> first matmul: split it across the
            # sync + (idle) swdge queues so it lands ~1us earlier
            nc.sync.dma_start(x16_t[0][:, 0:512], x16_d[:, 0:512])
            nc.gpsimd.dma_start(x16_t[0][:, 512:NB], x16_d[:, 512:NB])
            for p in range(1, 4):
                eng = nc.sync if p % 2 == 0 else nc.scalar
                eng.dma_start(x16_t[p][:], x16_d[:, p * NB:(p + 1) * NB])
            nc.vector.memset(ones8[:], 1.0)
            nc.vector.memset(ones16[:], 1.0)

            def x16_chunk(j):
                p, r = divmod(j * C, NB)
                return x16_t[p][:, r:r + C]

            def qproj(b, on_dve):
                """q16 for block b: 2 fp16 matmuls + bias-adds split across
                DVE/ACT so both halves finish in parallel (gates first QK)."""
                for h in range(2):
                    qp = ps_sc.tile([C, 512], f32, tag="sc", name=f"qp{b}_{h}")
                    nc.tensor.matmul(qp[:], wqwt16,
                                     x16_t[b][:, h * 512:(h + 1) * 512],
                                     start=True, stop=True)
                    dst = q16_t[b][:, h * 512:(h + 1) * 512]
                    if (h == 0) == on_dve:
                        nc.vector.tensor_scalar(dst, qp[:], bqw, None, op0=ADD)
                    else:
                        nc.scalar.add(dst, qp[:], bqw)

            def vproj(g):
                """v chunks 4g..4g+3 (fp16 matmuls) -> v8 group g (fp8)."""
                vp = ps_sc.tile([C, 512], f32, tag="sc", name=f"vp{g}")
                for t in range(4):
                    nc.tensor.matmul(vp[:, t * C:(t + 1) * C],
                                     x16_chunk(4 * g + t), wvt16,
                                     start=True, stop=True)
                if g % 2 == 0:
                    nc.vector.tensor_scalar(v8_t[g][:], vp[:], bv16, None,
                                            op0=ADD)
                else:
                    nc.scalar.add(v8_t[g][:], vp[:], bv16)

            # exp engine per half-op: ACT exact exp vs DVE Schraudolph.
            ecnt = [0]

            def emit_exp(es_t, u, h, sc):
                dst = es_t[:, u, h * 512:(h + 1) * 512]
                on_act = (ecnt[0] * 5) % 9 < 5
                ecnt[0] += 1
                if on_act:
                    nc.scalar.activation(dst, sc[:], EXP, bias=0.0,
                                         scale=float(SCALE))
                else:
                    nc.vector.tensor_scalar(dst.bitcast(u8), sc[:],
                                            DVE_MULT, float(B5),
                                            op0=MUL, op1=ADD)

            def tail(b, pv, rs):
                n0 = b * NB
                last = b == NBLK - 1
                rb = work.tile([C, NB], f32, tag="rb", name=f"rb{b}")
                ep = work.tile([C, NB], f32, tag="ep", name=f"ep{b}")
                ost = work.tile([C, NB], f16, tag="ost", name=f"ost{b}")
                if last:
                    # exposed tail: quarter-grain DVE chain so each output
                    # quarter DMAs out while the next quarter computes
                    for q in range(4):
                        qsl = slice(q * 256, (q + 1) * 256)
                        nc.vector.reciprocal_approx_fast(out=rb[:, qsl],
                                                         in_=rs[:, qsl])
                        nc.vector.tensor_tensor(ep[:, qsl], pv[:, qsl],
                                                rb[:, qsl], op=MUL)
                        nc.vector.tensor_tensor(ost[:, qsl], ep[:, qsl],
                                                x16_t[b][:, qsl], op=ADD)
                        eng = nc.sync if q % 2 == 0 else nc.scalar
                        eng.dma_start(out_d[:, n0 + qsl.start:n0 + qsl.stop],
                                      ost[:, qsl])
                    return
                for h in range(2):
                    hsl = slice(h * 512, (h + 1) * 512)
                    nc.vector.reciprocal_approx_fast(out=rb[:, hsl],
                                                     in_=rs[:, hsl])
                    nc.vector.tensor_tensor(ep[:, hsl], pv[:, hsl], rb[:, hsl],
                                            op=MUL)
                    nc.gpsimd.tensor_tensor(ost[:, hsl], ep[:, hsl],
                                            x16_t[b][:, hsl], op=ADD)
                    eng = nc.sync if h == 0 else nc.scalar
                    eng.dma_start(out_d[:, n0 + hsl.start:n0 + hsl.stop],
                                  ost[:, hsl])

            qproj(0, on_dve=True)

            for b in range(NBLK):
                pv = ps_pv.tile([C, NB], f32, tag="pv", name=f"pv{b}")
                rs = ps_rs.tile([C, NB], f32, tag="rs", name=f"rs{b}")

                def flush_pv(pend):
                    # both PV halves share one v8-pair LDWEIGHTS, both RS
                    # halves share the ones8 load: 2 weight loads per pair
                    # instead of 4 (each reload stalls the PE ~160ns).
                    jp, et = pend
                    for h in range(2):
                        hsl = slice(h * 512, (h + 1) * 512)
                        nc.tensor.matmul(pv[:, hsl],
                                         v8_t[jp // 2][:, (jp % 2) * 2:(jp % 2) * 2 + 2, :],
                                         et[:, :, hsl],
                                         start=(jp == 0), stop=(jp == NPAIR - 1),
                                         perf_mode=DR)
                    for h in range(2):
                        hsl = slice(h * 512, (h + 1) * 512)
                        nc.tensor.matmul(rs[:, hsl], ones8[:],
                                         et[:, :, hsl],
                                         start=(jp == 0), stop=(jp == NPAIR - 1),
                                         perf_mode=DR)

                es_t = None
                pendq = []   # PV/RS deferred and flushed TWO pairs at a
                # time: fewer fp16<->fp8 mode transitions in the PE stream
                # (each first-matmul-after-transition costs ~150ns), and the
                # in-order PE always has fresh QK work before exp-gated reads
                for j in range(MCH):
                    if b == 0 and j % 4 == 0:
                        vproj(j // 4)
                    if j % 2 == 0:
                        es_t = es_pool.tile([C, 2, NB], f8e5, tag="es",
                                            name=f"es{b}_{j // 2}")
                    for h in range(2):
                        sc = ps_sc.tile([C, 512], f32, tag="sc",
                                        name=f"sc{b}_{j}_{h}")
                        nc.tensor.matmul(
                            sc[:], x16_chunk(j),
                            q16_t[b][:, h * 512:(h + 1) * 512],
                            start=True, stop=True)
                        emit_exp(es_t, j % 2, h, sc)
                    if j % 2 == 1:
                        pendq.append((j // 2, es_t))
                        if len(pendq) == 2:
                            for pend in pendq:
                                flush_pv(pend)
                            pendq = []
                    if j == 19 and b < NBLK - 1:
                        qproj(b + 1, on_dve=(b % 2 == 0))
                for pend in pendq:
                    flush_pv(pend)
                tail(b, pv, rs)

    nc.finalize()
    return nc


def _get_nc():
    if "nc" not in _cache:
        _cache["nc"] = _build_nc()
    return _cache["nc"]


def make_in_maps(x, Wq, bq, Wk, bk, Wv, bv):
    x = np.asarray(x, dtype=np.float32)
    B = x.shape[0]
    wks = np.asarray(Wk, np.float32).sum(axis=0)            # [C]
    Wqw = np.asarray(Wq, np.float32) * wks[:, None]
    bqw = np.asarray(bq, np.float32) * wks
    w16 = np.ascontiguousarray(np.concatenate(
        [Wqw.T.astype(np.float16), np.asarray(Wv, np.float32).T.astype(np.float16)],
        axis=1))
    smalls = np.ascontiguousarray(
        np.stack([bqw, np.asarray(bv, np.float32)], axis=1).astype(np.float32))

    in_maps = []
    for i in range(B):
        xf = np.ascontiguousarray(x[i].reshape(C, N))
        in_maps.append({
            "x16": xf.astype(np.float16),
            "w16": w16, "smalls": smalls,
        })
    return in_maps


def kernel(x, Wq, bq, Wk, bk, Wv, bv, _trace=False, _tmpdir=None):
    from concourse.bass_utils import run_bass_kernel_spmd

    x = np.asarray(x, dtype=np.float32)
    B, c, H, W = x.shape
    assert (c, H * W) == (C, N), (c, H, W)
    in_maps = make_in_maps(x, Wq, bq, Wk, bk, Wv, bv)
    nc = _get_nc()
    res = run_bass_kernel_spmd(nc, in_maps, core_ids=list(range(B)),
                               trace=_trace, tmpdir=_tmpdir)
    out = np.stack([
        np.asarray(res.results[i]["out"]).astype(np.float32).reshape(C, H, W)
        for i in range(B)
    ])
    if _trace:
        _cache["last_result"] = res
    return out
